# revision 2
# baseline (speedup 1.0000x reference)
"""Trainium2 Bass kernel for nn_ConvexMLPBlock (v2: fp32r mm1 + overlap).

Reference computation (B=64, HW=196, D=768, E=256, C=10):
    S[b,h,e]  = (x[b,h,:] @ ag_w[e,:] + ag_b[e]) > 0          (sign patterns)
    z[b,h,p]  = x[b,h,:] @ lm_w[p,:]        (p = e*C + c)
    preds[b,c] = sum_{h,e} S[b,h,e] * z[b,h,e,c] / (HW*E)

Restructured to avoid materializing z (49 GFLOP -> ~10 GFLOP):
    G_b[e,d]   = sum_h S[b,h,e] * x[b,h,d]                    (per-batch masked moment)
    preds[b,c] = (1/(HW*E)) * sum_{e,d} G_b[e,d] * W[e,c,d]   (W = lm_w.reshape(E,C,D))

Sharding: data-parallel over B across the 8 NeuronCores (8 batches/core).

v2 changes vs the 70.9us v1:
  mm1 runs in ONE fp32r pass (HW-measured: ~13.2 effective mantissa bits,
  110ns/MM at FD=196 with changing stationary) instead of fp16 + fp8-DR
  cross terms (3 passes). Sign-flip error budget: ~150 flips -> ~1e-2 rel
  err, inside the 2e-2 gate. Saves ~8us PE and 0.8MB DMA.
  Chunks are per-batch (W1=196, NCH=8) so S, xn, and mm2 pipeline per
  batch: mm2 for batches 0..5 is emitted inside mm1's DMA-stall windows;
  batches 6,7 + finals interleave per-dt so only ~1.2us of finals trails
  the last wfin byte. The wfin dep-gate is gone (it starved the fabric);
  queue order alone paces everything. Final 1/(HW*E) scale is folded into
  mask/sel3 (1/224 each, fp16-normal range).

Per-core pipeline:
    mm1: S^T[e,t] = (agt32^T @ xt32 + b) > 0, fp32r, per-batch chunks.
    threshold: DVE tensor_scalar (psum + bias) > 0 -> bf16 0/1.
    PE-transpose S^T -> S natural per (batch, h-half, e-tile).
    mm2: G^T_b[d,e] contraction over h, fp16 (S exact; x rounded).
    finals: 96 matmuls (dt, g) accumulate into ONE [128, C*EG] PSUM tile;
    mask-mult + sel3 matmul + reduce finish it.
"""

import numpy as np

import concourse.bass as bass
import concourse.mybir as mybir
import concourse.tile as tile
from concourse.bass_utils import run_bass_kernel_spmd

# Problem constants (hardcoded per contract).
B = 64
HW = 196
D = 768
E = 256
C = 10
NCORES = 8
BL = B // NCORES          # local batches per core
T = BL * HW               # local tokens = 1568
KT = D // 128             # 6 d-tiles
ET = E // 128             # 2 e-tiles
W1 = HW                   # mm1 moving-dim chunk = one batch (196 tokens)
NCH = BL                  # 8 chunks
EG = 16                   # e's per final-stage group
NG = E // EG              # 16 groups

FP32 = mybir.dt.float32
FP32R = mybir.dt.float32r
BF16 = mybir.dt.bfloat16
FP16 = mybir.dt.float16

NWARM = 26                # PE warm-up matmuls bridging the DMA prologue


def _patched_drain_and_barrier(self, tick_clock, wait_clock):
    """This toolchain's walrus rejects >1 sync-wait on CTRL-class (Drain)
    instructions. Split the tail drain's global-clock waits across multiple
    single-wait drains. Semantics preserved: SP observes every DMA-queue
    semaphore before the all-engine barrier."""
    drain_inst = self.nc.sync.drain()
    wait_clock.add_sem_waits(
        drain_inst.ins, tile.ScopedClock({None: tick_clock.global_clock})
    )
    si = drain_inst.ins.sync_info
    if si is not None and si.on_wait is not None and len(si.on_wait) > 1:
        waits = list(si.on_wait)
        drain_inst.ins.sync_info = mybir.SyncInfo(
            on_wait=[waits[0]], on_update=list(si.on_update or [])
        )
        for w in waits[1:]:
            extra = self.nc.sync.drain()
            extra.ins.sync_info = mybir.SyncInfo(on_wait=[w], on_update=[])

    self.nc.all_engine_barrier()
    assert self.sems is not None
    popped = self.nc._tile_sem_poison_stack.pop()
    assert popped is self._sem_poison
    self.nc.clear_and_free_semaphores(list(self.sems.allocated().values()))
    self.nc.all_engine_barrier()


tile.TileContext._drain_and_barrier = _patched_drain_and_barrier


def _split_multiwait_json(bj: bytes) -> bytes:
    """Walrus in this toolchain accepts at most one sync-wait per instruction.
    For any instruction with N>1 waits, hoist N-1 waits onto same-engine NoOps
    inserted immediately before it. Engines execute program-order, so for
    compute instructions this is semantically identical; for DMAs it
    conservatively blocks the issuing engine instead of the queue."""
    import json

    m = json.loads(bj)
    changed = False
    for fn in m["functions"]:
        for bb in fn["blocks"]:
            new_insts = []
            for inst in bb["instructions"]:
                si = inst.get("sync_info")
                ow = (si or {}).get("on_wait") or []
                if len(ow) > 1:
                    for j, w in enumerate(ow[:-1]):
                        new_insts.append(
                            {
                                "name": f"{inst['name']}__w{j}",
                                "opcode": "NoOp",
                                "engine": inst["engine"],
                                "ins": [],
                                "outs": [],
                                "sync_info": {"on_update": [], "on_wait": [w]},
                            }
                        )
                    si["on_wait"] = [ow[-1]]
                    changed = True
                new_insts.append(inst)
            bb["instructions"] = new_insts
    if not changed:
        return bj
    return json.dumps(m).encode()


_orig_to_json_bytes = bass.Bass.to_json_bytes


def _patched_to_json_bytes(self, *a, **k):
    return _split_multiwait_json(_orig_to_json_bytes(self, *a, **k))


bass.Bass.to_json_bytes = _patched_to_json_bytes


def build_program():
    nc = bass.Bass()

    # host layouts put the SBUF partition dim first so each load is ONE
    # contiguous-iteration DMA.
    xt32_d = nc.dram_tensor("xt32", (128, NCH, KT, W1), FP32R,
                            kind="ExternalInput").ap()
    agt32_d = nc.dram_tensor("agt32", (128, KT, E), FP32R,
                             kind="ExternalInput").ap()
    agb_d = nc.dram_tensor("agb", (128, ET), FP32, kind="ExternalInput").ap()
    # xn_a[r, b, d] = x[b, r, d]; xn_b[r, b, d] = x[b, 128+r, d]
    xn_a_d = nc.dram_tensor("xn_a", (128, BL, D), FP16, kind="ExternalInput").ap()
    xn_b_d = nc.dram_tensor("xn_b", (HW - 128, BL, D), FP16,
                            kind="ExternalInput").ap()
    # wfin[dp, dt, g, c, el] = lm_w[(g*EG+el)*C+c, dt*128+dp]
    wfin_d = nc.dram_tensor("wfin", (128, KT, NG, C, EG), FP16,
                            kind="ExternalInput").ap()
    # mask[b*EG+ep, c, e] = (e == ep)/224; selects diagonal e-blocks
    mask_d = nc.dram_tensor("mask", (128, C, EG), FP16, kind="ExternalInput").ap()
    # sel3[b*EG+ep, bp] = (b == bp)/224; partition-sums per batch
    sel3_d = nc.dram_tensor("sel3", (128, BL), FP16, kind="ExternalInput").ap()
    ident_d = nc.dram_tensor("ident", (128, 128), BF16, kind="ExternalInput").ap()
    preds_o = nc.dram_tensor("preds_o", (BL, C), FP32, kind="ExternalOutput").ap()

    from contextlib import ExitStack
    with tile.TileContext(nc) as tc, ExitStack() as _es:
        xt_p = _es.enter_context(tc.tile_pool(name="xt_p", bufs=1))
        agt_p = _es.enter_context(tc.tile_pool(name="agt_p", bufs=1))
        small_p = _es.enter_context(tc.tile_pool(name="small_p", bufs=1))
        st_p = _es.enter_context(tc.tile_pool(name="st_p", bufs=1))
        sn_p = _es.enter_context(tc.tile_pool(name="sn_p", bufs=1))
        xn_p = _es.enter_context(tc.tile_pool(name="xn_p", bufs=1))
        gt_p = _es.enter_context(tc.tile_pool(name="gt_p", bufs=1))
        wfin_p = _es.enter_context(tc.tile_pool(name="wfin_p", bufs=1))
        out_p = _es.enter_context(tc.tile_pool(name="out_p", bufs=1))
        m_p = _es.enter_context(tc.tile_pool(name="m_p", bufs=1))
        ps1 = _es.enter_context(tc.tile_pool(name="ps1", bufs=2, space="PSUM"))
        pst = _es.enter_context(tc.tile_pool(name="pst", bufs=2, space="PSUM"))
        ps2 = _es.enter_context(tc.tile_pool(name="ps2", bufs=3, space="PSUM"))
        psF = _es.enter_context(tc.tile_pool(name="psF", bufs=1, space="PSUM"))

        # ---- persistent tiles ----
        agt32_sb = agt_p.tile([128, KT, E], FP32R, tag="agt32")
        agb_sb = small_p.tile([128, ET], FP32, tag="agb")
        xt32_sb = [xt_p.tile([128, KT, W1], FP32R, tag=f"xt{n}",
                             name=f"xt32_sb{n}") for n in range(NCH)]
        xn_a_sb = [xn_p.tile([128, 2, D], FP16, tag=f"xna{n}",
                             name=f"xn_a_sb{n}") for n in range(BL // 2)]
        xn_b_sb = [xn_p.tile([HW - 128, 2, D], FP16, tag=f"xnb{n}",
                             name=f"xn_b_sb{n}") for n in range(BL // 2)]
        wfin_sb = [wfin_p.tile([128, NG, C, EG], FP16, tag=f"wfin{dt}",
                               name=f"wfin_sb{dt}") for dt in range(KT)]
        mask_sb = m_p.tile([128, C, EG], FP16, tag="mask")
        sel3_sb = m_p.tile([128, BL], FP16, tag="sel3")
        ident_sb = small_p.tile([128, 128], BF16, tag="ident")
        st_sb = [st_p.tile([128, T], BF16, tag=f"st{et}",
                           name=f"st_sb{et}") for et in range(ET)]
        sn_sb = [
            [sn_p.tile([128, E], FP16, tag=f"sn{b}_{ht}",
                       name=f"sn_sb{b}_{ht}") for ht in range(2)]
            for b in range(BL)
        ]
        gt_sb = [gt_p.tile([128, NG, BL, EG], FP16, tag=f"gt{dt}",
                           name=f"gt_sb{dt}")
                 for dt in range(KT)]

        # ---- DMA issue, by queue, in consumption order ----
        # sync (SP) and scalar (ACT) are the two HWDGE queues (~180 GB/s
        # each when both saturated); gpsimd carries only tiny early loads.
        # xt chunks alternate queues so mm1's per-batch pacing is ~1.7us;
        # xn pairs are interleaved mid-stream so mm2 for early batches can
        # run inside mm1's DMA stalls; wfin (per-dt sixths) lands last,
        # just ahead of each finals-dt group. No dep gates: queue FIFO
        # order paces everything.
        nc.gpsimd.dma_start(agb_sb[:], agb_d[:, :])
        nc.gpsimd.dma_start(ident_sb[:], ident_d[:, :])
        # sync: c0, c2, xn01, c4, c6, xn45, wf0, wf2, wf4, wf5
        nc.sync.dma_start(xt32_sb[0][:], xt32_d[:, 0])
        nc.sync.dma_start(xt32_sb[2][:], xt32_d[:, 2])
        nc.sync.dma_start(xn_a_sb[0][:], xn_a_d[:, 0:2, :])
        nc.sync.dma_start(xn_b_sb[0][:], xn_b_d[:, 0:2, :])
        nc.sync.dma_start(xt32_sb[4][:], xt32_d[:, 4])
        nc.sync.dma_start(xt32_sb[6][:], xt32_d[:, 6])
        nc.sync.dma_start(xn_a_sb[2][:], xn_a_d[:, 4:6, :])
        nc.sync.dma_start(xn_b_sb[2][:], xn_b_d[:, 4:6, :])
        nc.sync.dma_start(wfin_sb[0][:], wfin_d[:, 0])
        nc.sync.dma_start(wfin_sb[2][:], wfin_d[:, 2])
        nc.sync.dma_start(wfin_sb[4][:], wfin_d[:, 4])
        nc.sync.dma_start(wfin_sb[5][:], wfin_d[:, 5])
        # scalar: agt32, c1, c3, xn23, c5, c7, xn67, wf1, wf3, mask, sel3
        nc.scalar.dma_start(agt32_sb[:], agt32_d[:, :])
        nc.scalar.dma_start(xt32_sb[1][:], xt32_d[:, 1])
        nc.scalar.dma_start(xt32_sb[3][:], xt32_d[:, 3])
        nc.scalar.dma_start(xn_a_sb[1][:], xn_a_d[:, 2:4, :])
        nc.scalar.dma_start(xn_b_sb[1][:], xn_b_d[:, 2:4, :])
        nc.scalar.dma_start(xt32_sb[5][:], xt32_d[:, 5])
        nc.scalar.dma_start(xt32_sb[7][:], xt32_d[:, 7])
        nc.scalar.dma_start(xn_a_sb[3][:], xn_a_d[:, 6:8, :])
        nc.scalar.dma_start(xn_b_sb[3][:], xn_b_d[:, 6:8, :])
        nc.scalar.dma_start(wfin_sb[1][:], wfin_d[:, 1])
        nc.scalar.dma_start(wfin_sb[3][:], wfin_d[:, 3])
        nc.scalar.dma_start(mask_sb[:], mask_d[:, :, :])
        nc.scalar.dma_start(sel3_sb[:], sel3_d[:, :])

        # ---- PE warm-up: HAM boosts the PE clock (1.2 -> 2.4 GHz) only
        # after a few us of sustained matmul activity. Fill the DMA wait.
        warm_src = small_p.tile([128, W1], FP16, tag="warm_src")
        nc.vector.memset(warm_src[:], 0.0)
        warm_w = small_p.tile([128, 128], FP16, tag="warm_w")
        nc.vector.memset(warm_w[:], 0.0)
        for wi in range(NWARM):
            wps = ps1.tile([128, W1], FP32, tag="ps1", name=f"warm_ps{wi}")
            nc.tensor.matmul(wps[:], warm_w[:], warm_src[:], start=True,
                             stop=True)

        def emit_mm2(b):
            """mm2 for batch b: G^T_b[d,e] per dt, copied into gt tiles."""
            for dt in range(KT):
                dsl = slice(dt * 128, (dt + 1) * 128)
                pg = ps2.tile([128, E], FP32, tag="ps2", name=f"ps2_{b}_{dt}")
                nc.tensor.matmul(pg[:], xn_a_sb[b // 2][:, b % 2, dsl],
                                 sn_sb[b][0][:], start=True, stop=False)
                nc.tensor.matmul(pg[:], xn_b_sb[b // 2][0:HW - 128, b % 2, dsl],
                                 sn_sb[b][1][0:HW - 128, :],
                                 start=False, stop=True)
                if b % 2 == 0:
                    nc.vector.tensor_copy(gt_sb[dt][:, :, b, :], pg[:])
                else:
                    nc.scalar.copy(gt_sb[dt][:, :, b, :], pg[:])

        # ---- mm1 (per-batch chunks) + transposes, with mm2 for early
        # batches emitted into mm1's DMA-stall windows ----
        for b in range(NCH):
            for et in range(ET):
                esl = slice(et * 128, (et + 1) * 128)
                ps = ps1.tile([128, W1], FP32, tag="ps1",
                              name=f"ps1_{et}_{b}")
                for kt in range(KT):
                    nc.tensor.matmul(
                        ps[:],
                        agt32_sb[:, kt, esl],
                        xt32_sb[b][:, kt, :],
                        start=(kt == 0),
                        stop=(kt == KT - 1),
                    )
                nc.vector.tensor_scalar(
                    st_sb[et][:, b * HW:(b + 1) * HW],
                    ps[:],
                    agb_sb[:, et:et + 1],
                    0.0,
                    mybir.AluOpType.add,
                    mybir.AluOpType.is_gt,
                )
            # transpose S^T -> S natural for this batch
            for ht in range(2):
                w = 128 if ht == 0 else HW - 128
                for et in range(ET):
                    pt = pst.tile([128, 128], BF16, tag="pst",
                                  name=f"pst_{b}_{ht}_{et}")
                    nc.tensor.transpose(
                        pt[0:w, :],
                        st_sb[et][:, b * HW + ht * 128:
                                  b * HW + ht * 128 + w],
                        ident_sb[:],
                    )
                    esl = slice(et * 128, (et + 1) * 128)
                    if (b + et) % 2 == 0:
                        nc.vector.tensor_copy(sn_sb[b][ht][0:w, esl],
                                              pt[0:w, :])
                    else:
                        nc.scalar.copy(sn_sb[b][ht][0:w, esl],
                                       pt[0:w, :])
            # early-batch mm2 slotted into mm1's DMA stalls
            if b == 3:
                emit_mm2(0)
                emit_mm2(1)
            elif b == 5:
                emit_mm2(2)
                emit_mm2(3)
            elif b == 7:
                emit_mm2(4)
                emit_mm2(5)

        # ---- mm2 for last batches + finals, interleaved per dt ----
        # All 96 final matmuls accumulate into ONE psum tile pf; the
        # diagonal e-mask commutes with the (dt, g) sum.
        pf = psF.tile([128, C, EG], FP32, tag="pf")
        ip = 0
        for dt in range(KT):
            dsl = slice(dt * 128, (dt + 1) * 128)
            for b in (6, 7):
                pg = ps2.tile([128, E], FP32, tag="ps2", name=f"ps2_{b}_{dt}")
                nc.tensor.matmul(pg[:], xn_a_sb[b // 2][:, b % 2, dsl],
                                 sn_sb[b][0][:], start=True, stop=False)
                nc.tensor.matmul(pg[:], xn_b_sb[b // 2][0:HW - 128, b % 2, dsl],
                                 sn_sb[b][1][0:HW - 128, :],
                                 start=False, stop=True)
                if b % 2 == 0:
                    nc.vector.tensor_copy(gt_sb[dt][:, :, b, :], pg[:])
                else:
                    nc.scalar.copy(gt_sb[dt][:, :, b, :], pg[:])
            for g in range(NG):
                nc.tensor.matmul(
                    pf[:],
                    gt_sb[dt][:, g, :, :],
                    wfin_sb[dt][:, g, :, :],
                    start=(ip == 0),
                    stop=(ip == KT * NG - 1),
                )
                ip += 1

        # ---- tail: mask diagonal (scale folded), partition-sum, reduce ----
        msb = out_p.tile([128, C, EG], FP16, tag="msb")
        nc.vector.tensor_tensor(msb[:], pf[:], mask_sb[:],
                                mybir.AluOpType.mult)
        pf2 = ps1.tile([BL, C, EG], FP32, tag="ps1", name="pf2")
        nc.tensor.matmul(pf2[:], sel3_sb[:], msb[:], start=True, stop=True)
        red_sb = out_p.tile([BL, C], FP32, tag="red")
        nc.vector.tensor_reduce(red_sb[:], pf2[:], mybir.AxisListType.X,
                                mybir.AluOpType.add)
        nc.sync.dma_start(preds_o[:, :], red_sb[:])

    return nc


_program_cache = {}

CONFIG = {"mm1": "fp32r", "mm2": "fp16", "fin": "accum-interleaved"}


def _get_program(**kw):
    key = tuple(sorted(kw.items()))
    if key not in _program_cache:
        _program_cache[key] = build_program()
    return _program_cache[key]


def make_in_maps(x, ag_w, ag_b, lm_w, cfg=None):
    import ml_dtypes

    x = np.ascontiguousarray(np.asarray(x, dtype=np.float32))
    ag_w = np.asarray(ag_w, dtype=np.float32)
    ag_b = np.asarray(ag_b, dtype=np.float32)
    lm_w = np.asarray(lm_w, dtype=np.float32)

    agb = np.ascontiguousarray(ag_b.reshape(ET, 128).T)
    agt32 = np.ascontiguousarray(
        ag_w.T.reshape(KT, 128, E).transpose(1, 0, 2))     # (128, KT, E)

    # wfin[dp, dt, g, c, el] = lm_w[(g*EG+el)*C+c, dt*128+dp]
    wfin = np.ascontiguousarray(
        lm_w.T.reshape(KT, 128, NG, EG, C)
        .transpose(1, 0, 2, 4, 3)
        .astype(np.float16)
    )
    ident = np.eye(128, dtype=ml_dtypes.bfloat16)
    SCL = 1.0 / np.sqrt(float(HW * E))   # split scale: fp16-normal range
    ep = np.arange(128) % EG
    mask = np.ascontiguousarray(
        ((ep[:, None, None] == np.arange(EG)[None, None, :])
         * np.full((128, C, EG), SCL)).astype(np.float16))
    bidx = np.arange(128) // EG
    sel3 = ((bidx[:, None] == np.arange(BL)[None, :]) * SCL).astype(np.float16)

    common = {"agb": agb, "agt32": agt32, "wfin": wfin,
              "ident": ident, "mask": mask, "sel3": sel3}

    in_maps = []
    for i in range(NCORES):
        xs = x[i * BL:(i + 1) * BL].reshape(T, D)
        m = dict(common)
        xr = xs.reshape(BL, HW, D).astype(np.float16)
        m["xn_a"] = np.ascontiguousarray(xr[:, 0:128, :].transpose(1, 0, 2))
        m["xn_b"] = np.ascontiguousarray(xr[:, 128:HW, :].transpose(1, 0, 2))
        m["xt32"] = np.ascontiguousarray(
            xs.T.reshape(KT, 128, NCH, W1).transpose(1, 2, 0, 3))
        in_maps.append(m)
    return in_maps


def kernel(x, ag_w, ag_b, lm_w):
    in_maps = make_in_maps(x, ag_w, ag_b, lm_w)
    nc = _get_program()
    res = run_bass_kernel_spmd(nc, in_maps, core_ids=list(range(NCORES)))
    preds = np.concatenate(
        [res.results[i]["preds_o"] for i in range(NCORES)], axis=0
    )
    return np.ascontiguousarray(preds.astype(np.float32))


# revision 4
# speedup vs baseline: 1.0747x; 1.0747x over previous
"""Trainium2 Bass kernel for nn_ConvexMLPBlock (v2: fp32r mm1 + overlap).

Reference computation (B=64, HW=196, D=768, E=256, C=10):
    S[b,h,e]  = (x[b,h,:] @ ag_w[e,:] + ag_b[e]) > 0          (sign patterns)
    z[b,h,p]  = x[b,h,:] @ lm_w[p,:]        (p = e*C + c)
    preds[b,c] = sum_{h,e} S[b,h,e] * z[b,h,e,c] / (HW*E)

Restructured to avoid materializing z (49 GFLOP -> ~10 GFLOP):
    G_b[e,d]   = sum_h S[b,h,e] * x[b,h,d]                    (per-batch masked moment)
    preds[b,c] = (1/(HW*E)) * sum_{e,d} G_b[e,d] * W[e,c,d]   (W = lm_w.reshape(E,C,D))

Sharding: data-parallel over B across the 8 NeuronCores (8 batches/core).

v2 changes vs the 70.9us v1:
  mm1 runs in ONE fp32r pass (HW-measured: ~13.2 effective mantissa bits,
  110ns/MM at FD=196 with changing stationary) instead of fp16 + fp8-DR
  cross terms (3 passes). Sign-flip error budget: ~150 flips -> ~1e-2 rel
  err, inside the 2e-2 gate. Saves ~8us PE and 0.8MB DMA.
  Chunks are per-batch (W1=196, NCH=8) so S, xn, and mm2 pipeline per
  batch: mm2 for batches 0..5 is emitted inside mm1's DMA-stall windows;
  batches 6,7 + finals interleave per-dt so only ~1.2us of finals trails
  the last wfin byte. The wfin dep-gate is gone (it starved the fabric);
  queue order alone paces everything. Final 1/(HW*E) scale is folded into
  mask/sel3 (1/224 each, fp16-normal range).

Per-core pipeline:
    mm1: S^T[e,t] = (agt32^T @ xt32 + b) > 0, fp32r, per-batch chunks.
    threshold: DVE tensor_scalar (psum + bias) > 0 -> bf16 0/1.
    PE-transpose S^T -> S natural per (batch, h-half, e-tile).
    mm2: G^T_b[d,e] contraction over h, fp16 (S exact; x rounded).
    finals: 96 matmuls (dt, g) accumulate into ONE [128, C*EG] PSUM tile;
    mask-mult + sel3 matmul + reduce finish it.
"""

import numpy as np

import concourse.bass as bass
import concourse.mybir as mybir
import concourse.tile as tile
from concourse.bass_utils import run_bass_kernel_spmd

# Problem constants (hardcoded per contract).
B = 64
HW = 196
D = 768
E = 256
C = 10
NCORES = 8
BL = B // NCORES          # local batches per core
T = BL * HW               # local tokens = 1568
KT = D // 128             # 6 d-tiles
ET = E // 128             # 2 e-tiles
W1 = HW                   # mm1 moving-dim chunk = one batch (196 tokens)
NCH = BL                  # 8 chunks
EG = 16                   # e's per final-stage group
NG = E // EG              # 16 groups

FP32 = mybir.dt.float32
FP32R = mybir.dt.float32r
BF16 = mybir.dt.bfloat16
FP16 = mybir.dt.float16

NWARM = 40                # PE warm-up matmuls bridging the DMA prologue


def _patched_drain_and_barrier(self, tick_clock, wait_clock):
    """This toolchain's walrus rejects >1 sync-wait on CTRL-class (Drain)
    instructions. Split the tail drain's global-clock waits across multiple
    single-wait drains. Semantics preserved: SP observes every DMA-queue
    semaphore before the all-engine barrier."""
    drain_inst = self.nc.sync.drain()
    wait_clock.add_sem_waits(
        drain_inst.ins, tile.ScopedClock({None: tick_clock.global_clock})
    )
    si = drain_inst.ins.sync_info
    if si is not None and si.on_wait is not None and len(si.on_wait) > 1:
        waits = list(si.on_wait)
        drain_inst.ins.sync_info = mybir.SyncInfo(
            on_wait=[waits[0]], on_update=list(si.on_update or [])
        )
        for w in waits[1:]:
            extra = self.nc.sync.drain()
            extra.ins.sync_info = mybir.SyncInfo(on_wait=[w], on_update=[])

    self.nc.all_engine_barrier()
    assert self.sems is not None
    popped = self.nc._tile_sem_poison_stack.pop()
    assert popped is self._sem_poison
    self.nc.clear_and_free_semaphores(list(self.sems.allocated().values()))
    self.nc.all_engine_barrier()


tile.TileContext._drain_and_barrier = _patched_drain_and_barrier


def _split_multiwait_json(bj: bytes) -> bytes:
    """Walrus in this toolchain accepts at most one sync-wait per instruction.
    For any instruction with N>1 waits, hoist N-1 waits onto same-engine NoOps
    inserted immediately before it. Engines execute program-order, so for
    compute instructions this is semantically identical; for DMAs it
    conservatively blocks the issuing engine instead of the queue."""
    import json

    m = json.loads(bj)
    changed = False
    for fn in m["functions"]:
        for bb in fn["blocks"]:
            new_insts = []
            for inst in bb["instructions"]:
                si = inst.get("sync_info")
                ow = (si or {}).get("on_wait") or []
                if len(ow) > 1:
                    for j, w in enumerate(ow[:-1]):
                        new_insts.append(
                            {
                                "name": f"{inst['name']}__w{j}",
                                "opcode": "NoOp",
                                "engine": inst["engine"],
                                "ins": [],
                                "outs": [],
                                "sync_info": {"on_update": [], "on_wait": [w]},
                            }
                        )
                    si["on_wait"] = [ow[-1]]
                    changed = True
                new_insts.append(inst)
            bb["instructions"] = new_insts
    if not changed:
        return bj
    return json.dumps(m).encode()


_orig_to_json_bytes = bass.Bass.to_json_bytes


def _patched_to_json_bytes(self, *a, **k):
    return _split_multiwait_json(_orig_to_json_bytes(self, *a, **k))


bass.Bass.to_json_bytes = _patched_to_json_bytes


def build_program():
    nc = bass.Bass()

    # host layouts put the SBUF partition dim first so each load is ONE
    # contiguous-iteration DMA.
    xt32_d = nc.dram_tensor("xt32", (128, NCH, KT, W1), FP32R,
                            kind="ExternalInput").ap()
    agt32_d = nc.dram_tensor("agt32", (128, KT, E), FP32R,
                             kind="ExternalInput").ap()
    agb_d = nc.dram_tensor("agb", (128, ET), FP32, kind="ExternalInput").ap()
    # xn_a[r, b, d] = x[b, r, d]; xn_b[r, b, d] = x[b, 128+r, d]
    xn_a_d = nc.dram_tensor("xn_a", (128, BL, D), FP16, kind="ExternalInput").ap()
    xn_b_d = nc.dram_tensor("xn_b", (HW - 128, BL, D), FP16,
                            kind="ExternalInput").ap()
    # wfin[dp, dt, g, c, el] = lm_w[(g*EG+el)*C+c, dt*128+dp]
    wfin_d = nc.dram_tensor("wfin", (128, KT, NG, C, EG), FP16,
                            kind="ExternalInput").ap()
    # mask[b*EG+ep, c, e] = (e == ep)/224; selects diagonal e-blocks
    mask_d = nc.dram_tensor("mask", (128, C, EG), FP16, kind="ExternalInput").ap()
    # sel3[b*EG+ep, bp] = (b == bp)/224; partition-sums per batch
    sel3_d = nc.dram_tensor("sel3", (128, BL), FP16, kind="ExternalInput").ap()
    ident_d = nc.dram_tensor("ident", (128, 128), BF16, kind="ExternalInput").ap()
    preds_o = nc.dram_tensor("preds_o", (BL, C), FP32, kind="ExternalOutput").ap()

    from contextlib import ExitStack
    with tile.TileContext(nc) as tc, ExitStack() as _es:
        xt_p = _es.enter_context(tc.tile_pool(name="xt_p", bufs=1))
        agt_p = _es.enter_context(tc.tile_pool(name="agt_p", bufs=1))
        small_p = _es.enter_context(tc.tile_pool(name="small_p", bufs=1))
        st_p = _es.enter_context(tc.tile_pool(name="st_p", bufs=1))
        sn_p = _es.enter_context(tc.tile_pool(name="sn_p", bufs=1))
        xn_p = _es.enter_context(tc.tile_pool(name="xn_p", bufs=1))
        gt_p = _es.enter_context(tc.tile_pool(name="gt_p", bufs=1))
        wfin_p = _es.enter_context(tc.tile_pool(name="wfin_p", bufs=1))
        out_p = _es.enter_context(tc.tile_pool(name="out_p", bufs=1))
        m_p = _es.enter_context(tc.tile_pool(name="m_p", bufs=1))
        ps1 = _es.enter_context(tc.tile_pool(name="ps1", bufs=2, space="PSUM"))
        pst = _es.enter_context(tc.tile_pool(name="pst", bufs=2, space="PSUM"))
        ps2 = _es.enter_context(tc.tile_pool(name="ps2", bufs=3, space="PSUM"))
        psF = _es.enter_context(tc.tile_pool(name="psF", bufs=1, space="PSUM"))

        # ---- persistent tiles ----
        agt32_sb = agt_p.tile([128, KT, E], FP32R, tag="agt32")
        agb_sb = small_p.tile([128, ET], FP32, tag="agb")
        xt32_sb = [xt_p.tile([128, KT, W1], FP32R, tag=f"xt{n}",
                             name=f"xt32_sb{n}") for n in range(NCH)]
        xn_a_sb = [xn_p.tile([128, 2, D], FP16, tag=f"xna{n}",
                             name=f"xn_a_sb{n}") for n in range(BL // 2)]
        xn_b_sb = [xn_p.tile([HW - 128, 2, D], FP16, tag=f"xnb{n}",
                             name=f"xn_b_sb{n}") for n in range(BL // 2)]
        wfin_sb = [wfin_p.tile([128, NG, C, EG], FP16, tag=f"wfin{dt}",
                               name=f"wfin_sb{dt}") for dt in range(KT)]
        mask_sb = m_p.tile([128, C, EG], FP16, tag="mask")
        sel3_sb = m_p.tile([128, BL], FP16, tag="sel3")
        ident_sb = small_p.tile([128, 128], BF16, tag="ident")
        st_sb = [st_p.tile([128, T], BF16, tag=f"st{et}",
                           name=f"st_sb{et}") for et in range(ET)]
        sn_sb = [
            [sn_p.tile([128, E], FP16, tag=f"sn{b}_{ht}",
                       name=f"sn_sb{b}_{ht}") for ht in range(2)]
            for b in range(BL)
        ]
        gt_sb = [gt_p.tile([128, NG, BL, EG], FP16, tag=f"gt{dt}",
                           name=f"gt_sb{dt}")
                 for dt in range(KT)]

        # ---- DMA issue, by queue, in consumption order ----
        # THREE queues: sync (SP, HWDGE), scalar (ACT, HWDGE), gpsimd
        # (SWDGE). The HWDGE ring is shallow: when an engine issues more
        # dma_starts than the ring holds, the LATER issue instructions
        # block the issuing engine until earlier transfers drain. The ACT
        # engine also runs half the PSUM-drain copies, so it gets only a
        # handful of issues; SP does nothing else, so it takes the bulk;
        # the otherwise-idle gpsimd queue carries late-needed mid-size
        # loads as a third bandwidth lane. No dep gates: queue FIFO order
        # paces everything.
        # sync: c0, c2, xn01, c4, c6, xn45, wf0, wf2, wf4, wf5 (6.2MB)
        nc.sync.dma_start(xt32_sb[0][:], xt32_d[:, 0])
        nc.sync.dma_start(xt32_sb[2][:], xt32_d[:, 2])
        nc.sync.dma_start(xn_a_sb[0][:], xn_a_d[:, 0:2, :])
        nc.sync.dma_start(xn_b_sb[0][:], xn_b_d[:, 0:2, :])
        nc.sync.dma_start(xt32_sb[4][:], xt32_d[:, 4])
        nc.sync.dma_start(xt32_sb[6][:], xt32_d[:, 6])
        nc.sync.dma_start(xn_a_sb[2][:], xn_a_d[:, 4:6, :])
        nc.sync.dma_start(xn_b_sb[2][:], xn_b_d[:, 4:6, :])
        nc.sync.dma_start(wfin_sb[0][:], wfin_d[:, 0])
        nc.sync.dma_start(wfin_sb[2][:], wfin_d[:, 2])
        nc.sync.dma_start(wfin_sb[4][:], wfin_d[:, 4])
        nc.sync.dma_start(wfin_sb[5][:], wfin_d[:, 5])
        # scalar: agt32, c1, c3, c5, c7 (3.2MB, 5 issues only)
        nc.scalar.dma_start(agt32_sb[:], agt32_d[:, :])
        nc.scalar.dma_start(xt32_sb[1][:], xt32_d[:, 1])
        nc.scalar.dma_start(xt32_sb[3][:], xt32_d[:, 3])
        nc.scalar.dma_start(xt32_sb[5][:], xt32_d[:, 5])
        nc.scalar.dma_start(xt32_sb[7][:], xt32_d[:, 7])
        # gpsimd: agb, ident, xn23, xn67, wf1, wf3, mask, sel3 (2.6MB)
        nc.gpsimd.dma_start(agb_sb[:], agb_d[:, :])
        nc.gpsimd.dma_start(ident_sb[:], ident_d[:, :])
        nc.gpsimd.dma_start(xn_a_sb[1][:], xn_a_d[:, 2:4, :])
        nc.gpsimd.dma_start(xn_b_sb[1][:], xn_b_d[:, 2:4, :])
        nc.gpsimd.dma_start(xn_a_sb[3][:], xn_a_d[:, 6:8, :])
        nc.gpsimd.dma_start(xn_b_sb[3][:], xn_b_d[:, 6:8, :])
        nc.gpsimd.dma_start(wfin_sb[1][:], wfin_d[:, 1])
        nc.gpsimd.dma_start(wfin_sb[3][:], wfin_d[:, 3])
        nc.gpsimd.dma_start(mask_sb[:], mask_d[:, :, :])
        nc.gpsimd.dma_start(sel3_sb[:], sel3_d[:, :])

        # ---- PE warm-up: HAM boosts the PE clock (1.2 -> 2.4 GHz) only
        # after a few us of sustained matmul activity. Fill the DMA wait.
        warm_src = small_p.tile([128, W1], FP16, tag="warm_src")
        nc.vector.memset(warm_src[:], 0.0)
        warm_w = small_p.tile([128, 128], FP16, tag="warm_w")
        nc.vector.memset(warm_w[:], 0.0)
        for wi in range(NWARM):
            wps = ps1.tile([128, W1], FP32, tag="ps1", name=f"warm_ps{wi}")
            nc.tensor.matmul(wps[:], warm_w[:], warm_src[:], start=True,
                             stop=True)

        def emit_mm2(b):
            """mm2 for batch b: G^T_b[d,e] per dt, copied into gt tiles."""
            for dt in range(KT):
                dsl = slice(dt * 128, (dt + 1) * 128)
                pg = ps2.tile([128, E], FP32, tag="ps2", name=f"ps2_{b}_{dt}")
                nc.tensor.matmul(pg[:], xn_a_sb[b // 2][:, b % 2, dsl],
                                 sn_sb[b][0][:], start=True, stop=False)
                nc.tensor.matmul(pg[:], xn_b_sb[b // 2][0:HW - 128, b % 2, dsl],
                                 sn_sb[b][1][0:HW - 128, :],
                                 start=False, stop=True)
                if b % 2 == 0:
                    nc.vector.tensor_copy(gt_sb[dt][:, :, b, :], pg[:])
                else:
                    nc.scalar.copy(gt_sb[dt][:, :, b, :], pg[:])

        # ---- mm1 (per-batch chunks) + transposes, with mm2 for early
        # batches emitted into mm1's DMA-stall windows ----
        for b in range(NCH):
            for et in range(ET):
                esl = slice(et * 128, (et + 1) * 128)
                ps = ps1.tile([128, W1], FP32, tag="ps1",
                              name=f"ps1_{et}_{b}")
                for kt in range(KT):
                    nc.tensor.matmul(
                        ps[:],
                        agt32_sb[:, kt, esl],
                        xt32_sb[b][:, kt, :],
                        start=(kt == 0),
                        stop=(kt == KT - 1),
                    )
                nc.vector.tensor_scalar(
                    st_sb[et][:, b * HW:(b + 1) * HW],
                    ps[:],
                    agb_sb[:, et:et + 1],
                    0.0,
                    mybir.AluOpType.add,
                    mybir.AluOpType.is_gt,
                )
            # transpose S^T -> S natural for this batch
            for ht in range(2):
                w = 128 if ht == 0 else HW - 128
                for et in range(ET):
                    pt = pst.tile([128, 128], BF16, tag="pst",
                                  name=f"pst_{b}_{ht}_{et}")
                    nc.tensor.transpose(
                        pt[0:w, :],
                        st_sb[et][:, b * HW + ht * 128:
                                  b * HW + ht * 128 + w],
                        ident_sb[:],
                    )
                    esl = slice(et * 128, (et + 1) * 128)
                    if (b + et) % 2 == 0:
                        nc.vector.tensor_copy(sn_sb[b][ht][0:w, esl],
                                              pt[0:w, :])
                    else:
                        nc.scalar.copy(sn_sb[b][ht][0:w, esl],
                                       pt[0:w, :])
            # early-batch mm2 slotted into mm1's DMA stalls
            if b == 3:
                emit_mm2(0)
                emit_mm2(1)
            elif b == 5:
                emit_mm2(2)
                emit_mm2(3)
            elif b == 7:
                emit_mm2(4)
                emit_mm2(5)

        # ---- mm2 for last batches + finals, interleaved per dt ----
        # All 96 final matmuls accumulate into ONE psum tile pf; the
        # diagonal e-mask commutes with the (dt, g) sum.
        pf = psF.tile([128, C, EG], FP32, tag="pf")
        ip = 0
        for dt in range(KT):
            dsl = slice(dt * 128, (dt + 1) * 128)
            for b in (6, 7):
                pg = ps2.tile([128, E], FP32, tag="ps2", name=f"ps2_{b}_{dt}")
                nc.tensor.matmul(pg[:], xn_a_sb[b // 2][:, b % 2, dsl],
                                 sn_sb[b][0][:], start=True, stop=False)
                nc.tensor.matmul(pg[:], xn_b_sb[b // 2][0:HW - 128, b % 2, dsl],
                                 sn_sb[b][1][0:HW - 128, :],
                                 start=False, stop=True)
                if b % 2 == 0:
                    nc.vector.tensor_copy(gt_sb[dt][:, :, b, :], pg[:])
                else:
                    nc.scalar.copy(gt_sb[dt][:, :, b, :], pg[:])
            for g in range(NG):
                nc.tensor.matmul(
                    pf[:],
                    gt_sb[dt][:, g, :, :],
                    wfin_sb[dt][:, g, :, :],
                    start=(ip == 0),
                    stop=(ip == KT * NG - 1),
                )
                ip += 1

        # ---- tail: mask diagonal (scale folded), partition-sum, reduce ----
        msb = out_p.tile([128, C, EG], FP16, tag="msb")
        nc.vector.tensor_tensor(msb[:], pf[:], mask_sb[:],
                                mybir.AluOpType.mult)
        pf2 = ps1.tile([BL, C, EG], FP32, tag="ps1", name="pf2")
        nc.tensor.matmul(pf2[:], sel3_sb[:], msb[:], start=True, stop=True)
        red_sb = out_p.tile([BL, C], FP32, tag="red")
        nc.vector.tensor_reduce(red_sb[:], pf2[:], mybir.AxisListType.X,
                                mybir.AluOpType.add)
        nc.sync.dma_start(preds_o[:, :], red_sb[:])

    return nc


_program_cache = {}

CONFIG = {"mm1": "fp32r", "mm2": "fp16", "fin": "accum-interleaved"}


def _get_program(**kw):
    key = tuple(sorted(kw.items()))
    if key not in _program_cache:
        _program_cache[key] = build_program()
    return _program_cache[key]


def make_in_maps(x, ag_w, ag_b, lm_w, cfg=None):
    import ml_dtypes

    x = np.ascontiguousarray(np.asarray(x, dtype=np.float32))
    ag_w = np.asarray(ag_w, dtype=np.float32)
    ag_b = np.asarray(ag_b, dtype=np.float32)
    lm_w = np.asarray(lm_w, dtype=np.float32)

    agb = np.ascontiguousarray(ag_b.reshape(ET, 128).T)
    agt32 = np.ascontiguousarray(
        ag_w.T.reshape(KT, 128, E).transpose(1, 0, 2))     # (128, KT, E)

    # wfin[dp, dt, g, c, el] = lm_w[(g*EG+el)*C+c, dt*128+dp]
    wfin = np.ascontiguousarray(
        lm_w.T.reshape(KT, 128, NG, EG, C)
        .transpose(1, 0, 2, 4, 3)
        .astype(np.float16)
    )
    ident = np.eye(128, dtype=ml_dtypes.bfloat16)
    SCL = 1.0 / np.sqrt(float(HW * E))   # split scale: fp16-normal range
    ep = np.arange(128) % EG
    mask = np.ascontiguousarray(
        ((ep[:, None, None] == np.arange(EG)[None, None, :])
         * np.full((128, C, EG), SCL)).astype(np.float16))
    bidx = np.arange(128) // EG
    sel3 = ((bidx[:, None] == np.arange(BL)[None, :]) * SCL).astype(np.float16)

    common = {"agb": agb, "agt32": agt32, "wfin": wfin,
              "ident": ident, "mask": mask, "sel3": sel3}

    in_maps = []
    for i in range(NCORES):
        xs = x[i * BL:(i + 1) * BL].reshape(T, D)
        m = dict(common)
        xr = xs.reshape(BL, HW, D).astype(np.float16)
        m["xn_a"] = np.ascontiguousarray(xr[:, 0:128, :].transpose(1, 0, 2))
        m["xn_b"] = np.ascontiguousarray(xr[:, 128:HW, :].transpose(1, 0, 2))
        m["xt32"] = np.ascontiguousarray(
            xs.T.reshape(KT, 128, NCH, W1).transpose(1, 2, 0, 3))
        in_maps.append(m)
    return in_maps


def kernel(x, ag_w, ag_b, lm_w):
    in_maps = make_in_maps(x, ag_w, ag_b, lm_w)
    nc = _get_program()
    res = run_bass_kernel_spmd(nc, in_maps, core_ids=list(range(NCORES)))
    preds = np.concatenate(
        [res.results[i]["preds_o"] for i in range(NCORES)], axis=0
    )
    return np.ascontiguousarray(preds.astype(np.float32))
